# revision 19
# baseline (speedup 1.0000x reference)
"""Grouped Conv1d (B=4, T=512, G=129, F=96 -> O=96, K=3, pad=1) on 8 trn2 cores.

Sharding: 129 groups = 16 full groups per core + group 128 split across all
8 cores by (batch b = core//2, T-half = core%2).  SPMD: every core runs the
identical program on its own slice.

PE strategy: the F*K=288-row contraction per (group, batch) is split into
nine 32-row chunks.  Groups are processed 4 at a time, each pinned to its
own 32-partition row range (group g' at partitions 32g'): four chunk
matmuls on four different row groups stream CONCURRENTLY through the PE
array (tile_position row tiling) - ~4 columns/cycle vs 1 for a single
96-row matmul.  Each PSUM tile is written from exactly one row position
(multi-position accumulation into one PSUM region aborts at runtime).
Batches of 4 groups run in two b-pair halves so the 16 in-flight
accumulators fit the 8 PSUM banks with zero-stall FIFO recycling.  x is
fed as float8e3 (e3m4: max rel err ~1.4e-2 incl. subnormals), weights
fp16, accumulate fp32.

DMA strategy: all big transfers span 128 partitions evenly (balanced
across the 16 SDMA engines).  x arrives per batch as 0.26MB column pieces
on the SP ring, outputs batch 8 units (0.77MB) per store on the ACT ring
at a steady ~3.8us cadence, small tail/bias tensors via SWDGE.  Warm-up
matmuls on zeroed scratch ramp the PE clock (HAM) while the prologue DMA
lands.
"""

from contextlib import ExitStack

import numpy as np
import ml_dtypes

import concourse.bass as bass
import concourse.mybir as mybir
import concourse.tile as tile
from concourse import bacc
from concourse.bass_utils import run_bass_kernel_spmd

B, T, G, F, O, K = 4, 512, 129, 96, 96, 3
NCORES = 8
GPC = 16          # full groups per core
NB = 4            # x/out DMA + compute batches (4 groups each)
GPB = 4           # groups per batch
TP = T + 2        # padded T
TE = T // 2       # tail-group T chunk per core
TEP = TE + 2
SLOT = B * TP     # x free-dim elems per (group, f-strip): 4*514
NWARM = 8         # PE warm-up matmuls (HAM ramp) on zeroed scratch


def build_program():
    nc = bacc.Bacc("TRN2", target_bir_lowering=False, debug=False,
                   num_devices=NCORES)

    f32 = mybir.dt.float32
    f16 = mybir.dt.float16
    f8 = mybir.dt.float8e3

    xm = nc.dram_tensor("xm", [NB, 128, 3 * SLOT], f8, kind="ExternalInput")
    wt = nc.dram_tensor("wt", [128, NB * 3 * K * O], f16, kind="ExternalInput")
    xe = nc.dram_tensor("xe", [F, TEP], f8, kind="ExternalInput")
    wte = nc.dram_tensor("wte", [F, K * O], f16, kind="ExternalInput")
    om = nc.dram_tensor("om", [NB, O, GPB * B * T], f16, kind="ExternalOutput")
    oe = nc.dram_tensor("oe", [O, TE], f16, kind="ExternalOutput")

    with ExitStack() as ctx:
        tc = ctx.enter_context(tile.TileContext(nc))
        wpool = ctx.enter_context(tc.tile_pool(name="w", bufs=1))
        xpool = ctx.enter_context(tc.tile_pool(name="x", bufs=3))
        opool = ctx.enter_context(tc.tile_pool(name="o", bufs=3))
        pspool = ctx.enter_context(tc.tile_pool(name="ps", bufs=8, space="PSUM"))

        w_sb = wpool.tile([128, NB * 3 * K * O], f16)
        xe_sb = wpool.tile([F, TEP], f8)
        wte_sb = wpool.tile([F, K * O], f16)
        xdum = wpool.tile([128, 256], f8)
        wdum = wpool.tile([128, O], f16)

        # PE warm-up: ramp HAM to full clock while the prologue DMA lands.
        nc.vector.memset(xdum[:], 0.0)
        nc.vector.memset(wdum[:], 0.0)
        psdum = pspool.tile([O, 256], f32, tag="ps", name="psdum")
        for i in range(NWARM):
            nc.tensor.matmul(psdum[:], wdum[:], xdum[:],
                             start=(i == 0), stop=(i == NWARM - 1))

        # prologue: first batch's x + all weights ride the SP ring in need
        # order; small tensors via SWDGE.
        kw = 3 * K * O
        nc.gpsimd.dma_start(xe_sb[:], xe[:])
        nc.gpsimd.dma_start(wte_sb[:], wte[:])

        x_tiles = {}

        def load_x(a, pieces):
            x_sb = xpool.tile([128, 3 * SLOT], f8, tag="x", name=f"x{a}")
            x_tiles[a] = x_sb
            for p in range(pieces):
                lo = p * (3 * SLOT) // pieces
                hi = (p + 1) * (3 * SLOT) // pieces
                nc.sync.dma_start(x_sb[:, lo:hi], xm[a][:, lo:hi])

        # batch 0 in 6 pieces aligned to consumption order: (fs, b01) for
        # half 0 then (fs, b23) for half 1 - compute starts ~0.5us after
        # the first 0.13MB piece lands.
        x_sb0 = xpool.tile([128, 3 * SLOT], f8, tag="x", name="x0")
        x_tiles[0] = x_sb0
        # interleave per-fs weight and x pieces in exact consumption
        # order: half 0's s=0..8 needs only b0 columns of each f-strip
        for fs in range(3):
            nc.sync.dma_start(w_sb[:, fs * K * O:(fs + 1) * K * O],
                              wt[:, fs * K * O:(fs + 1) * K * O])
            nc.sync.dma_start(
                x_sb0[:, fs * SLOT:fs * SLOT + TP],
                xm[0][:, fs * SLOT:fs * SLOT + TP])
        for fs in range(3):
            nc.sync.dma_start(
                x_sb0[:, fs * SLOT + TP:fs * SLOT + 2 * TP],
                xm[0][:, fs * SLOT + TP:fs * SLOT + 2 * TP])
        for fs in range(3):
            nc.sync.dma_start(
                x_sb0[:, fs * SLOT + 2 * TP:(fs + 1) * SLOT],
                xm[0][:, fs * SLOT + 2 * TP:(fs + 1) * SLOT])
        nc.sync.dma_start(w_sb[:, kw:], wt[:, kw:])    # remaining weights
        load_x(1, 2)

        for a in range(NB):
            if a + 2 < NB:
                load_x(a + 2, 1)
            x_sb = x_tiles.pop(a)
            for h in range(2):                         # b-pair halves
                o_half = opool.tile([O, GPB * 2 * T], f16, tag="o",
                                    name=f"o{a}_{h}")
                pss = {}
                for s in range(18):
                    j, c = s // 9, s % 9               # b-index, chunk
                    fs, kk = c // 3, c % 3
                    bb = 2 * h + j
                    for gl in range(4):
                        if c == 0:
                            pss[(gl, j)] = pspool.tile(
                                [O, T], f32, tag="ps", name=f"ps{a}{h}{gl}{j}")
                        nc.tensor.matmul(
                            pss[(gl, j)][:],
                            w_sb[32 * gl:32 * gl + 32,
                                 ((a * 3 + fs) * K + kk) * O:
                                 ((a * 3 + fs) * K + kk + 1) * O],
                            x_sb[32 * gl:32 * gl + 32,
                                 fs * SLOT + bb * TP + kk:
                                 fs * SLOT + bb * TP + kk + T],
                            start=(c == 0), stop=(c == 8),
                            tile_position=(32 * gl, 0),
                        )
                    if c == 8:
                        # the j-th b of every lane just closed: drain (plain
                        # fp32->fp16 copies, bias is added host-side) in
                        # half-T pieces split across DVE + ACT, then store
                        # the whole quad (0.38MB) on an alternating ring.
                        final = (a == NB - 1 and h == 1 and j == 1)
                        SPL = 256
                        for gl in range(4):
                            c0 = (j * GPB + gl) * T
                            nc.vector.tensor_copy(
                                o_half[:, c0:c0 + SPL],
                                pss[(gl, j)][:, :SPL])
                            nc.scalar.add(
                                o_half[:, c0 + SPL:c0 + T],
                                pss[(gl, j)][:, SPL:], 0.0)
                            if final:
                                # last quad: store per group as it drains on
                                # the two HWDGE rings (both engines free now)
                                eng = nc.sync if gl % 2 == 0 else nc.scalar
                                q0 = (h * 2 + j) * GPB * T
                                eng.dma_start(
                                    om[a][:, q0 + gl * T:q0 + (gl + 1) * T],
                                    o_half[:, (j * GPB + gl) * T:
                                           (j * GPB + gl + 1) * T])
                        if not final:
                            # store issue on the idle SP queue; never the
                            # drain engines
                            nc.sync.dma_start(
                                om[a][:, (h * 2 + j) * GPB * T:
                                      (h * 2 + j + 1) * GPB * T],
                                o_half[:, j * GPB * T:(j + 1) * GPB * T])

            if a == 1:
                # tail group (g=128): tiny; runs once mid-kernel
                ps = pspool.tile([O, TE], f32, tag="ps", name="pstail")
                for kk in range(K):
                    nc.tensor.matmul(
                        ps[:],
                        wte_sb[:, kk * O:(kk + 1) * O],
                        xe_sb[:, kk:kk + TE],
                        start=(kk == 0), stop=(kk == K - 1),
                    )
                oe_sb = wpool.tile([O, TE], f16)
                nc.vector.tensor_copy(oe_sb[:], ps[:])
                nc.gpsimd.dma_start(oe[:], oe_sb[:])

    nc.finalize()
    return nc


def shard_inputs(x, weight, bias):
    x = np.ascontiguousarray(x, dtype=np.float32)
    weight = np.ascontiguousarray(weight, dtype=np.float32)
    bias = np.ascontiguousarray(bias, dtype=np.float32)

    xp = np.pad(x, ((0, 0), (1, 1), (0, 0), (0, 0)))          # [B, TP, G, F]
    xt = np.ascontiguousarray(xp.transpose(2, 3, 0, 1)).astype(
        ml_dtypes.float8_e3m4)                                # [G, F, B, TP]
    wtr = weight.astype(np.float16)                           # [G, O, F, K]

    in_maps = []
    for c in range(NCORES):
        g0 = c * GPC
        b_c, t0 = c // 2, (c % 2) * TE
        # x: group g'=g%4 of batch a=g//4 at partitions 32g'; its 3
        # f-strips are the 3 column slots.
        xc = xt[g0:g0 + GPC].reshape(NB, GPB, 3, 32, B, TP)   # a,g',fs,i,b,t
        xc = xc.transpose(0, 1, 3, 2, 4, 5)                   # a,g',i,fs,b,t
        xm_c = np.ascontiguousarray(xc.reshape(NB, 128, 3 * SLOT))
        # w: same partition mapping; cols = (a, fs, k, o)
        wc = wtr[g0:g0 + GPC].transpose(0, 2, 3, 1)           # [16, F, K, O]
        wc = wc.reshape(NB, GPB, 3, 32, K, O)                 # a,g',fs,i,k,o
        wc = wc.transpose(1, 3, 0, 2, 4, 5)                   # g',i,a,fs,k,o
        wt_c = np.ascontiguousarray(wc.reshape(128, NB * 3 * K * O))
        in_maps.append({
            "xm": xm_c,
            "wt": wt_c,
            "xe": np.ascontiguousarray(xt[G - 1, :, b_c, t0:t0 + TEP]),
            "wte": np.ascontiguousarray(
                wtr[G - 1].transpose(1, 2, 0).reshape(F, K * O)),
        })
    return in_maps


def unshard_outputs(results):
    out = np.empty((B, T, G, O), dtype=np.float32)
    for c in range(NCORES):
        om = results[c]["om"].astype(np.float32)       # [NB, O, GPB*B*T]
        # om cols = (h, j, g', T) with b = 2h + j; bias is added host-side
        om = om.reshape(NB, O, 2, 2, GPB, T)           # a,o,h,j,g',t
        om = om.transpose(2, 3, 5, 0, 4, 1)            # h,j,t,a,g',o
        out[:, :, c * GPC:(c + 1) * GPC, :] = om.reshape(B, T, GPC, O)
        b_c, t0 = c // 2, (c % 2) * TE
        out[b_c, t0:t0 + TE, G - 1, :] = results[c]["oe"].astype(np.float32).T
    return out


def run(x, weight, bias, **run_kwargs):
    nc = build_program()
    in_maps = shard_inputs(x, weight, bias)
    res = run_bass_kernel_spmd(nc, in_maps, list(range(NCORES)), **run_kwargs)
    out = unshard_outputs(res.results)
    out += np.asarray(bias, dtype=np.float32)[None, None, :, :]
    return out, res


def kernel(x, weight, bias):
    out, _ = run(x, weight, bias)
    return out


# revision 21
# speedup vs baseline: 1.0392x; 1.0392x over previous
"""Grouped Conv1d (B=4, T=512, G=129, F=96 -> O=96, K=3, pad=1) on 8 trn2 cores.

Sharding: 129 groups = 16 full groups per core + group 128 split across all
8 cores by (batch b = core//2, T-half = core%2).  SPMD: every core runs the
identical program on its own slice.

PE strategy: the F*K=288-row contraction per (group, batch) is split into
nine 32-row chunks.  Groups are processed 4 at a time, each pinned to its
own 32-partition row range (group g' at partitions 32g'): four chunk
matmuls on four different row groups stream CONCURRENTLY through the PE
array (tile_position row tiling) - ~4 columns/cycle vs 1 for a single
96-row matmul.  Each PSUM tile is written from exactly one row position
(multi-position accumulation into one PSUM region aborts at runtime).
Batches of 4 groups run in two b-pair halves so the 16 in-flight
accumulators fit the 8 PSUM banks with zero-stall FIFO recycling.  x is
fed as float8e3 (e3m4: max rel err ~1.4e-2 incl. subnormals), weights
fp16, accumulate fp32.

DMA strategy: all big transfers span 128 partitions evenly (balanced
across the 16 SDMA engines).  x arrives per batch as 0.26MB column pieces
on the SP ring, outputs batch 8 units (0.77MB) per store on the ACT ring
at a steady ~3.8us cadence, small tail/bias tensors via SWDGE.  Warm-up
matmuls on zeroed scratch ramp the PE clock (HAM) while the prologue DMA
lands.
"""

from contextlib import ExitStack

import numpy as np
import ml_dtypes

import concourse.bass as bass
import concourse.mybir as mybir
import concourse.tile as tile
from concourse import bacc
from concourse.bass_utils import run_bass_kernel_spmd

B, T, G, F, O, K = 4, 512, 129, 96, 96, 3
NCORES = 8
GPC = 16          # full groups per core
NB = 4            # x/out DMA + compute batches (4 groups each)
GPB = 4           # groups per batch
TP = T + 2        # padded T
TE = T // 2       # tail-group T chunk per core
TEP = TE + 2
SLOT = B * TP     # x free-dim elems per (group, f-strip): 4*514
NWARM = 7         # PE warm-up matmuls (HAM ramp) on zeroed scratch


def build_program():
    nc = bacc.Bacc("TRN2", target_bir_lowering=False, debug=False,
                   num_devices=NCORES)

    f32 = mybir.dt.float32
    f16 = mybir.dt.float16
    f8 = mybir.dt.float8e3

    xm = nc.dram_tensor("xm", [NB, 128, 3 * SLOT], f8, kind="ExternalInput")
    wt = nc.dram_tensor("wt", [128, NB * 3 * K * O], f16, kind="ExternalInput")
    xe = nc.dram_tensor("xe", [F, TEP], f8, kind="ExternalInput")
    wte = nc.dram_tensor("wte", [F, K * O], f16, kind="ExternalInput")
    om = nc.dram_tensor("om", [NB, O, GPB * B * T], f16, kind="ExternalOutput")
    oe = nc.dram_tensor("oe", [O, TE], f16, kind="ExternalOutput")

    with ExitStack() as ctx:
        tc = ctx.enter_context(tile.TileContext(nc))
        wpool = ctx.enter_context(tc.tile_pool(name="w", bufs=1))
        xpool = ctx.enter_context(tc.tile_pool(name="x", bufs=3))
        opool = ctx.enter_context(tc.tile_pool(name="o", bufs=3))
        pspool = ctx.enter_context(tc.tile_pool(name="ps", bufs=8, space="PSUM"))

        w_sb = wpool.tile([128, NB * 3 * K * O], f16)
        xe_sb = wpool.tile([F, TEP], f8)
        wte_sb = wpool.tile([F, K * O], f16)
        xdum = wpool.tile([128, 256], f8)
        wdum = wpool.tile([128, O], f16)

        # PE warm-up: ramp HAM to full clock while the prologue DMA lands.
        nc.vector.memset(xdum[:], 0.0)
        nc.vector.memset(wdum[:], 0.0)
        psdum = pspool.tile([O, 256], f32, tag="ps", name="psdum")
        for i in range(NWARM):
            nc.tensor.matmul(psdum[:], wdum[:], xdum[:],
                             start=(i == 0), stop=(i == NWARM - 1))

        # prologue: first batch's x + all weights ride the SP ring in need
        # order; small tensors via SWDGE.
        kw = 3 * K * O
        nc.gpsimd.dma_start(xe_sb[:], xe[:])
        nc.gpsimd.dma_start(wte_sb[:], wte[:])

        x_tiles = {}

        def load_x(a, pieces):
            x_sb = xpool.tile([128, 3 * SLOT], f8, tag="x", name=f"x{a}")
            x_tiles[a] = x_sb
            for p in range(pieces):
                lo = p * (3 * SLOT) // pieces
                hi = (p + 1) * (3 * SLOT) // pieces
                nc.sync.dma_start(x_sb[:, lo:hi], xm[a][:, lo:hi])

        # batch 0 in 6 pieces aligned to consumption order: (fs, b01) for
        # half 0 then (fs, b23) for half 1 - compute starts ~0.5us after
        # the first 0.13MB piece lands.
        x_sb0 = xpool.tile([128, 3 * SLOT], f8, tag="x", name="x0")
        x_tiles[0] = x_sb0
        # minimal issue count (each dma_start costs ~650ns of engine
        # time): batch-0 weights, then batch-0 half-0 per-fs pieces, half-1
        # block, batch-1 half blocks with remaining weights between.
        nc.sync.dma_start(w_sb[:, :kw], wt[:, :kw])
        HB = 6 * TP                                    # columns per b-half
        for fs in range(3):
            nc.sync.dma_start(
                x_sb0[:, fs * 2 * TP:(fs + 1) * 2 * TP],
                xm[0][:, fs * 2 * TP:(fs + 1) * 2 * TP])
        nc.sync.dma_start(x_sb0[:, HB:], xm[0][:, HB:])
        x_sb1 = xpool.tile([128, 3 * SLOT], f8, tag="x", name="x1")
        x_tiles[1] = x_sb1
        nc.sync.dma_start(x_sb1[:, :HB], xm[1][:, :HB])
        nc.sync.dma_start(w_sb[:, kw:], wt[:, kw:])    # remaining weights
        nc.sync.dma_start(x_sb1[:, HB:], xm[1][:, HB:])

        for a in range(NB):
            if a + 2 < NB:
                load_x(a + 2, 1)
            x_sb = x_tiles.pop(a)
            for h in range(2):                         # b-pair halves
                o_half = opool.tile([O, GPB * 2 * T], f16, tag="o",
                                    name=f"o{a}_{h}")
                pss = {}
                for s in range(18):
                    j, c = s // 9, s % 9               # b-index, chunk
                    fs, kk = c // 3, c % 3
                    bb = 2 * h + j
                    for gl in range(4):
                        if c == 0:
                            pss[(gl, j)] = pspool.tile(
                                [O, T], f32, tag="ps", name=f"ps{a}{h}{gl}{j}")
                        nc.tensor.matmul(
                            pss[(gl, j)][:],
                            w_sb[32 * gl:32 * gl + 32,
                                 ((a * 3 + fs) * K + kk) * O:
                                 ((a * 3 + fs) * K + kk + 1) * O],
                            x_sb[32 * gl:32 * gl + 32,
                                 (h * 6 + fs * 2 + j) * TP + kk:
                                 (h * 6 + fs * 2 + j) * TP + kk + T],
                            start=(c == 0), stop=(c == 8),
                            tile_position=(32 * gl, 0),
                        )
                    if c == 8:
                        # the j-th b of every lane just closed: drain (plain
                        # fp32->fp16 copies, bias is added host-side) in
                        # half-T pieces split across DVE + ACT, then store
                        # the whole quad (0.38MB) on an alternating ring.
                        final = (a == NB - 1 and h == 1 and j == 1)
                        SPL = 256
                        for gl in range(4):
                            c0 = (j * GPB + gl) * T
                            nc.vector.tensor_copy(
                                o_half[:, c0:c0 + SPL],
                                pss[(gl, j)][:, :SPL])
                            nc.scalar.add(
                                o_half[:, c0 + SPL:c0 + T],
                                pss[(gl, j)][:, SPL:], 0.0)
                            if final:
                                # last quad: store per group as it drains on
                                # the two HWDGE rings (both engines free now)
                                eng = nc.sync if gl % 2 == 0 else nc.scalar
                                q0 = (h * 2 + j) * GPB * T
                                eng.dma_start(
                                    om[a][:, q0 + gl * T:q0 + (gl + 1) * T],
                                    o_half[:, (j * GPB + gl) * T:
                                           (j * GPB + gl + 1) * T])
                        if not final:
                            # store issue on the idle SP queue; never the
                            # drain engines
                            nc.sync.dma_start(
                                om[a][:, (h * 2 + j) * GPB * T:
                                      (h * 2 + j + 1) * GPB * T],
                                o_half[:, j * GPB * T:(j + 1) * GPB * T])

            if a == 1:
                # tail group (g=128): tiny; runs once mid-kernel
                ps = pspool.tile([O, TE], f32, tag="ps", name="pstail")
                for kk in range(K):
                    nc.tensor.matmul(
                        ps[:],
                        wte_sb[:, kk * O:(kk + 1) * O],
                        xe_sb[:, kk:kk + TE],
                        start=(kk == 0), stop=(kk == K - 1),
                    )
                oe_sb = wpool.tile([O, TE], f16)
                nc.vector.tensor_copy(oe_sb[:], ps[:])
                nc.gpsimd.dma_start(oe[:], oe_sb[:])

    nc.finalize()
    return nc


def shard_inputs(x, weight, bias):
    x = np.ascontiguousarray(x, dtype=np.float32)
    weight = np.ascontiguousarray(weight, dtype=np.float32)
    bias = np.ascontiguousarray(bias, dtype=np.float32)

    xp = np.pad(x, ((0, 0), (1, 1), (0, 0), (0, 0)))          # [B, TP, G, F]
    xt = np.ascontiguousarray(xp.transpose(2, 3, 0, 1)).astype(
        ml_dtypes.float8_e3m4)                                # [G, F, B, TP]
    wtr = weight.astype(np.float16)                           # [G, O, F, K]

    in_maps = []
    for c in range(NCORES):
        g0 = c * GPC
        b_c, t0 = c // 2, (c % 2) * TE
        # x: group g'=g%4 of batch a=g//4 at partitions 32g'; free dim
        # is [b-half][f-strip][b%2][t] so each compute half's columns are
        # one contiguous DMA range.
        xc = xt[g0:g0 + GPC].reshape(NB, GPB, 3, 32, 2, 2, TP)
        xc = xc.transpose(0, 1, 3, 4, 2, 5, 6)              # a,g',i,hb,fs,j,t
        xm_c = np.ascontiguousarray(xc.reshape(NB, 128, 3 * SLOT))
        # w: same partition mapping; cols = (a, fs, k, o)
        wc = wtr[g0:g0 + GPC].transpose(0, 2, 3, 1)           # [16, F, K, O]
        wc = wc.reshape(NB, GPB, 3, 32, K, O)                 # a,g',fs,i,k,o
        wc = wc.transpose(1, 3, 0, 2, 4, 5)                   # g',i,a,fs,k,o
        wt_c = np.ascontiguousarray(wc.reshape(128, NB * 3 * K * O))
        in_maps.append({
            "xm": xm_c,
            "wt": wt_c,
            "xe": np.ascontiguousarray(xt[G - 1, :, b_c, t0:t0 + TEP]),
            "wte": np.ascontiguousarray(
                wtr[G - 1].transpose(1, 2, 0).reshape(F, K * O)),
        })
    return in_maps


def unshard_outputs(results):
    out = np.empty((B, T, G, O), dtype=np.float32)
    for c in range(NCORES):
        om = results[c]["om"].astype(np.float32)       # [NB, O, GPB*B*T]
        # om cols = (h, j, g', T) with b = 2h + j; bias is added host-side
        om = om.reshape(NB, O, 2, 2, GPB, T)           # a,o,h,j,g',t
        om = om.transpose(2, 3, 5, 0, 4, 1)            # h,j,t,a,g',o
        out[:, :, c * GPC:(c + 1) * GPC, :] = om.reshape(B, T, GPC, O)
        b_c, t0 = c // 2, (c % 2) * TE
        out[b_c, t0:t0 + TE, G - 1, :] = results[c]["oe"].astype(np.float32).T
    return out


def run(x, weight, bias, **run_kwargs):
    nc = build_program()
    in_maps = shard_inputs(x, weight, bias)
    res = run_bass_kernel_spmd(nc, in_maps, list(range(NCORES)), **run_kwargs)
    out = unshard_outputs(res.results)
    out += np.asarray(bias, dtype=np.float32)[None, None, :, :]
    return out, res


def kernel(x, weight, bias):
    out, _ = run(x, weight, bias)
    return out
